# revision 14
# baseline (speedup 1.0000x reference)
"""KNN graph kernel for Trainium2 (8 NeuronCores, Bass/Tile).

Problem: per-batch 32-NN of 16384 queries against 16384 refs (B~4 batches,
both sorted by batch id).  Output matches the jax reference:
  e_ref  [M*32] int32  - nearest ref indices, ascending distance per query
  e_query[M*32] int32  - repeat(arange(M), 32)
  mask   [M*32] bool   - (q_z - r_z) >= -1e-5 per edge

Strategy (spatial windowing + quantized score/index packing):
  * Queries are grouped into 136 spatial cells of <=128 (per-batch y/z
    quantile split).  Each 128-query block only scans refs inside its
    cell's (y,z) bounding box expanded by RM=15 (<=1020 refs instead of
    the whole ~4096-ref batch).  A query's true 32NN ball fits in the
    window unless d_32 > RM; the host detects that exactly and repairs.
  * The tensor engine computes PSUM = T - c*d2 with T = 1.5*2^23: every
    PSUM value lands in [2^23, 2^24) where fp32 forces integer rounding,
    i.e. the score is quantized to 1/c for free.  ACT copies PSUM to SBUF
    subtracting T (exact); Pool (+ DVE for one chunk) adds idx/128
    (column-in-chunk index; exact while |m| <= 2^17 keeps the 24-bit
    total representable).  Packed values carry the quantized score AND
    the column index, so stage 1 is a SINGLE DVE max8 per 85-col chunk -
    no max_index pass, no on-device stage 2.
  * Host: decode candidates, rescore them exactly in f32 (same formula
    family as the reference), merge to top-32, and exactly recompute rows
    flagged for near-ties / chunk concealment / window-radius violations.
"""

import numpy as np

K = 32
P = 128            # queries per block (SBUF partitions)
W = 1024           # ref window cols (2 PSUM banks); last 4 always pad
CHUNK = 85
NCHUNK = 12        # 12*85 = 1020 data cols
NCAND = NCHUNK * 8
N_CORES = 8
NBLK = 17          # blocks per core -> 136 cells total
RM = 15.0          # window margin (covering-radius guarantee)
CSC = 256.0        # score scale: quantum = 1/CSC in d2 units
TBIG = 1.5 * 2**23  # 12582912; PSUM offset forcing integer quantization
MVALID = -131072.0  # candidates with m < MVALID (d2 > 512) are discarded
TAU_ORDER = 0.012  # adjacent-gap margin for exact-order trust (d2 units)
PAD_RR = 1.0e6     # |r|^2 for pad columns -> huge negative score

_CACHE = {}


def _np_exact_rows(q_rows_bxyz, ref_bxyz):
    """Reference-exact (f32) top-K ref indices for the given query rows."""
    rb, rx = ref_bxyz[:, 0], ref_bxyz[:, 1:4]
    qb, qx = q_rows_bxyz[:, 0], q_rows_bxyz[:, 1:4]
    d2 = (np.sum(qx * qx, axis=1)[:, None]
          + np.sum(rx * rx, axis=1)[None, :]
          - np.float32(2.0) * (qx @ rx.T)).astype(np.float32)
    d2[qb[:, None] != rb[None, :]] = np.inf
    return np.argsort(d2, axis=1, kind="stable")[:, :K].astype(np.int32)


def _np_fallback(ref_bxyz, query_bxyz):
    M = query_bxyz.shape[0]
    e_ref = np.empty((M, K), np.int32)
    step = 2048
    for s in range(0, M, step):
        e_ref[s:s + step] = _np_exact_rows(query_bxyz[s:s + step], ref_bxyz)
    return e_ref.reshape(-1)


def _build_program():
    import concourse.mybir as mybir
    import concourse.tile as tile
    from concourse import bacc

    nc = bacc.Bacc("TRN2", target_bir_lowering=False, debug=False, num_devices=1)
    f32, u32 = mybir.dt.float32, mybir.dt.uint32
    Copy = mybir.ActivationFunctionType.Copy

    qT = nc.dram_tensor("qT", [NBLK, 5, P], f32, kind="ExternalInput").ap()
    rslab = nc.dram_tensor("rslab", [NBLK, 5, W], f32, kind="ExternalInput").ap()
    c_val = nc.dram_tensor("c_val", [NBLK * P, NCAND], f32, kind="ExternalOutput").ap()

    HALF = NCHUNK * CHUNK // 2  # 510 cols per half (one PSUM bank each)
    DVE_COLS = 50               # trailing cols per half packed by DVE (balance)

    with tile.TileContext(nc) as tc:
        with tc.tile_pool(name="const", bufs=1) as cpool, \
             tc.tile_pool(name="qp", bufs=4) as qpool, \
             tc.tile_pool(name="rp", bufs=4) as rpool, \
             tc.tile_pool(name="tp", bufs=4) as tpool, \
             tc.tile_pool(name="kp", bufs=4) as kpool, \
             tc.tile_pool(name="cvp", bufs=4) as cvpool, \
             tc.tile_pool(name="ps", bufs=6, space="PSUM") as ppool:
            NREP = W // CHUNK + 1
            idxu = cpool.tile([P, NREP * CHUNK], u32)
            nc.gpsimd.iota(idxu[:], pattern=[[0, NREP], [1, CHUNK]],
                           base=0, channel_multiplier=0)
            idxrow = cpool.tile([P, W], f32)
            nc.vector.tensor_scalar_mul(idxrow[:], idxu[:, :W], 1.0 / 128.0)

            for blk in range(NBLK):
                qt = qpool.tile([5, P], f32)
                nc.sync.dma_start(out=qt[:], in_=qT[blk])
                rs = rpool.tile([5, W], f32)
                nc.sync.dma_start(out=rs[:], in_=rslab[blk])

                t = tpool.tile([P, W], f32)
                pk = kpool.tile([P, W], f32)
                cv = cvpool.tile([P, NCAND], f32)
                # two independent halves -> matmul/ACT/pack/max8 pipeline
                for h in range(2):
                    lo = HALF * h
                    ps = ppool.tile([P, HALF], f32)
                    nc.tensor.matmul(ps[:], qt[:], rs[:, lo:lo + HALF],
                                     start=True, stop=True)
                    nc.scalar.activation(t[:, lo:lo + HALF], ps[:],
                                         Copy, bias=-float(TBIG), scale=1.0)
                    csplit = lo + HALF - DVE_COLS
                    nc.gpsimd.tensor_add(pk[:, lo:csplit], t[:, lo:csplit],
                                         idxrow[:, lo:csplit])
                    nc.vector.tensor_add(pk[:, csplit:lo + HALF],
                                         t[:, csplit:lo + HALF],
                                         idxrow[:, csplit:lo + HALF])
                    for s in range(NCHUNK // 2 * h, NCHUNK // 2 * (h + 1)):
                        nc.vector.max(out=cv[:, 8 * s:8 * s + 8],
                                      in_=pk[:, CHUNK * s:CHUNK * (s + 1)])
                nc.sync.dma_start(out=c_val[blk * P:(blk + 1) * P], in_=cv[:])
    nc.compile()
    return nc


def _apportion(total, weights):
    """Split integer `total` proportionally to weights (largest remainder)."""
    w = np.asarray(weights, dtype=np.float64)
    if w.sum() <= 0:
        out = np.zeros(len(w), np.int64)
        if len(w):
            out[0] = total
        return out
    want = w / w.sum() * total
    out = np.floor(want).astype(np.int64)
    rem = int(total - out.sum())
    order = np.argsort(-(want - out))
    out[order[:rem]] += 1
    return out


def _plan_cells(nq_per_batch):
    """Apportion NBLK*N_CORES cells across batches, >= ceil(nq/P) each."""
    total = NBLK * N_CORES
    nq = np.asarray(nq_per_batch, dtype=np.int64)
    mins = -(-nq // P)
    if mins.sum() > total:
        return None
    extra = total - int(mins.sum())
    add = _apportion(extra, nq) if extra > 0 else np.zeros(len(nq), np.int64)
    ncells = mins + add
    ncells[nq == 0] = 0
    return ncells


def kernel(ref_bxyz: np.ndarray, query_bxyz: np.ndarray):
    ref_bxyz = np.ascontiguousarray(ref_bxyz, dtype=np.float32)
    query_bxyz = np.ascontiguousarray(query_bxyz, dtype=np.float32)
    M = query_bxyz.shape[0]
    e_query = np.repeat(np.arange(M, dtype=np.int32), K)

    def finish(e_ref_flat):
        direction = query_bxyz[e_query, 3] - ref_bxyz[e_ref_flat, 3]
        return e_ref_flat, e_query, (direction >= np.float32(-1e-5))

    rb, qb = ref_bxyz[:, 0], query_bxyz[:, 0]
    bids = np.unique(np.concatenate([rb, qb]))
    ok = (len(bids) <= NBLK * N_CORES
          and np.all(np.diff(rb) >= 0) and np.all(np.diff(qb) >= 0))
    if ok:
        r_starts = np.searchsorted(rb, bids, side="left")
        r_ends = np.searchsorted(rb, bids, side="right")
        q_starts = np.searchsorted(qb, bids, side="left")
        q_ends = np.searchsorted(qb, bids, side="right")
        nq_b = q_ends - q_starts
        nr_b = r_ends - r_starts
        ok = bool(np.all((nr_b >= K) | (nq_b == 0)))
        ncells = _plan_cells(nq_b) if ok else None
        ok = ok and ncells is not None
    if not ok:
        return finish(_np_fallback(ref_bxyz, query_bxyz))

    # ---- host prep: spatial cells, windows, slabs ----
    nb = len(bids)
    total_blocks = NBLK * N_CORES
    WDAT = NCHUNK * CHUNK
    qT_in = np.zeros((total_blocks, 5, P), np.float32)
    rs_in = np.empty((total_blocks, 5, W), np.float32)
    rs_in[:, 0:3, :] = 0.0
    rs_in[:, 3, :] = PAD_RR
    rs_in[:, 4, :] = 1.0
    qT_in[:, 4, :] = np.float32(TBIG)  # pad queries: q~ = 0

    win_idx = np.full((total_blocks, W), -1, np.int64)
    rm_blk = np.full(total_blocks, np.float64(RM))
    blk_q = [None] * total_blocks  # global query indices per block
    blk_box = np.zeros((total_blocks, 4), np.float64)  # ymin,ymax,zmin,zmax
    blk_bb = np.zeros((total_blocks, 4), np.float64)   # batch ref bounds y/z

    blk = 0
    cell_overflow = False
    for bi in range(nb):
        n_b = int(ncells[bi])
        if n_b == 0:
            continue
        qs_, qe_ = int(q_starts[bi]), int(q_ends[bi])
        rs_, re_ = int(r_starts[bi]), int(r_ends[bi])
        q_idx = np.arange(qs_, qe_)
        qy, qz = query_bxyz[qs_:qe_, 2], query_bxyz[qs_:qe_, 3]
        r_xyz = ref_bxyz[rs_:re_, 1:4]
        ry, rz = r_xyz[:, 1], r_xyz[:, 2]
        cx0 = float(r_xyz[:, 0].min() + r_xyz[:, 0].max()) / 2

        gy = 4 if n_b >= 8 else 1
        gz_per = _apportion(n_b, np.ones(gy))
        mq_per = _apportion(len(q_idx), gz_per)
        y_order = np.argsort(qy, kind="stable")
        gstart = 0
        for g in range(gy):
            gsel = y_order[gstart:gstart + int(mq_per[g])]
            gstart += int(mq_per[g])
            if len(gsel) == 0:
                continue
            z_order = gsel[np.argsort(qz[gsel], kind="stable")]
            for part in np.array_split(z_order, int(gz_per[g])):
                if len(part) == 0:
                    continue
                if len(part) > P:
                    cell_overflow = True
                    break
                cq = q_idx[part]  # global query ids, cell order
                cy, cz = qy[part], qz[part]
                ymin, ymax = float(cy.min()), float(cy.max())
                zmin, zmax = float(cz.min()), float(cz.max())
                need_y = np.maximum(ymin - ry, ry - ymax)
                need_z = np.maximum(zmin - rz, rz - zmax)
                need = np.maximum(np.maximum(need_y, need_z), 0.0)
                sel = np.nonzero(need <= RM)[0]
                rmb = RM
                if len(sel) > WDAT:
                    kept = np.argpartition(need, WDAT - 1)[:WDAT]
                    sel = np.sort(kept)
                    rmb = float(need[sel].max()) - 1e-4
                nw = len(sel)
                # stride-spread the window over all WDAT cols so each
                # chunk sees ~nw/NCHUNK refs (keeps chunk top-8 losses rare)
                spread = (np.arange(nw, dtype=np.int64) * WDAT) // max(nw, 1)
                win_idx[blk, spread] = rs_ + sel
                rm_blk[blk] = rmb
                blk_q[blk] = cq
                blk_box[blk] = (ymin, ymax, zmin, zmax)
                if re_ > rs_:
                    blk_bb[blk] = (ry.min(), ry.max(), rz.min(), rz.max())
                c0 = np.array([cx0, (ymin + ymax) / 2, (zmin + zmax) / 2],
                              np.float64)
                rt = (r_xyz[sel].astype(np.float64) - c0).astype(np.float32)
                rs_in[blk, 0:3, spread] = rt  # adv-index dim first: (nw, 3)
                rs_in[blk, 3, spread] = np.sum(
                    rt.astype(np.float64) ** 2, axis=1).astype(np.float32)
                qt_ = (query_bxyz[cq, 1:4].astype(np.float64) - c0).astype(
                    np.float32)
                nv = len(cq)
                qT_in[blk, 0:3, :nv] = (2.0 * CSC) * qt_.T
                qT_in[blk, 3, :] = np.float32(-CSC)
                qT_in[blk, 4, :nv] = (
                    TBIG - CSC * np.sum(qt_.astype(np.float64) ** 2, axis=1)
                ).astype(np.float32)
                blk += 1
            if cell_overflow:
                break
        if cell_overflow:
            break
    if cell_overflow:
        return finish(_np_fallback(ref_bxyz, query_bxyz))

    # ---- device ----
    if "nc" not in _CACHE:
        _CACHE["nc"] = _build_program()
    nc = _CACHE["nc"]
    from concourse.bass_utils import run_bass_kernel_spmd
    qT_c = qT_in.reshape(N_CORES, NBLK, 5, P)
    rs_c = rs_in.reshape(N_CORES, NBLK, 5, W)
    in_maps = [{"qT": qT_c[c], "rslab": rs_c[c]} for c in range(N_CORES)]
    _CACHE["last_in_maps"] = in_maps
    res = run_bass_kernel_spmd(nc, in_maps, list(range(N_CORES)))
    _CACHE["last_results"] = res

    cv = np.concatenate([res.results[c]["c_val"] for c in range(N_CORES)],
                        axis=0).reshape(total_blocks, P, NCAND)

    # ---- host post: decode, rescore, merge, repair ----
    blocks_list = [i for i in range(total_blocks) if blk_q[i] is not None]
    bsel = np.concatenate([np.full(len(blk_q[i]), i, np.int64)
                           for i in blocks_list])
    rsel = np.concatenate([np.arange(len(blk_q[i]), dtype=np.int64)
                           for i in blocks_list])
    q_flat = np.concatenate([blk_q[i] for i in blocks_list])  # [M] global qids

    p = cv[bsel, rsel].astype(np.float64)              # [M, NCAND]
    m = np.floor(p)
    idxl = np.rint((p - m) * 128.0).astype(np.int64)
    pos = (np.arange(NCAND) // 8)[None, :] * CHUNK + np.clip(idxl, 0, CHUNK - 1)
    valid = (m >= MVALID) & (idxl < CHUNK)
    d2q = -m / CSC                                     # device-quantized d2
    gidx = win_idx[bsel[:, None], pos]
    valid &= gidx >= 0
    gidx_c = np.where(valid, gidx, 0)

    qx_all = query_bxyz[:, 1:4]
    rx_all = ref_bxyz[:, 1:4]
    qq_all = np.sum(qx_all * qx_all, axis=1)           # f32, reference formula
    rr_all = np.sum(rx_all * rx_all, axis=1)

    dot = np.einsum("qd,qkd->qk", qx_all[q_flat], rx_all[gidx_c],
                    dtype=np.float32, casting="same_kind")
    d2x = (qq_all[q_flat][:, None] + rr_all[gidx_c]
           - np.float32(2.0) * dot).astype(np.float64)
    d2x[~valid] = np.inf

    near = valid & (d2q < 500.0)
    e_obs = float(np.abs(np.where(near, d2q - d2x, 0.0)).max())
    if e_obs > 1.0:
        return finish(_np_fallback(ref_bxyz, query_bxyz))
    kappa = 2.0 * e_obs + 2.0 / CSC + 1e-3

    order = np.lexsort((gidx_c, d2x), axis=-1)
    top = order[:, :K + 1]
    d2_sorted = np.take_along_axis(d2x, top, axis=1)
    ref_sorted = np.take_along_axis(gidx_c, top, axis=1)

    e_ref = np.empty((M, K), np.int32)
    e_ref[q_flat] = ref_sorted[:, :K].astype(np.int32)

    nvalid = valid.sum(axis=1)
    d2_32 = d2_sorted[:, K - 1]
    b_nv = nvalid < K + 1
    # per-query, per-axis-side window containment: the d_32 ball must fit in
    # the (y,z)-expanded cell box, except on sides where the window already
    # reaches the batch's ref extent (domain-clipped side: nothing to miss).
    d32r = np.sqrt(np.where(np.isfinite(d2_32), d2_32, 0.0)) + 1e-3
    qy_f = query_bxyz[q_flat, 2].astype(np.float64)
    qz_f = query_bxyz[q_flat, 3].astype(np.float64)
    box = blk_box[bsel]
    bb = blk_bb[bsel]
    rmb = rm_blk[bsel]
    ok = ((qy_f - d32r >= box[:, 0] - rmb) | (box[:, 0] - rmb <= bb[:, 0]))
    ok &= ((qy_f + d32r <= box[:, 1] + rmb) | (box[:, 1] + rmb >= bb[:, 1]))
    ok &= ((qz_f - d32r >= box[:, 2] - rmb) | (box[:, 2] - rmb <= bb[:, 2]))
    ok &= ((qz_f + d32r <= box[:, 3] + rmb) | (box[:, 3] + rmb >= bb[:, 3]))
    b_vio = (~ok) | ~np.isfinite(d2_32)
    b_tie = (np.diff(d2_sorted, axis=1) < TAU_ORDER).any(axis=1)
    ch8 = d2q[:, 7::8]
    b_con = (valid[:, 7::8] & (ch8 <= d2_32[:, None] + kappa)).any(axis=1)
    bad = b_nv | b_vio | b_tie | b_con
    _CACHE["sus"] = dict(nv=int(b_nv.sum()), vio=int(b_vio.sum()),
                         tie=int(b_tie.sum()), con=int(b_con.sum()),
                         e_obs=e_obs, kappa=kappa)
    _CACHE["dbg"] = dict(win_idx=win_idx, rm_blk=rm_blk, bsel=bsel,
                         d2_32=d2_32, d2q=d2q, valid=valid, nvalid=nvalid,
                         d2_sorted=d2_sorted, q_flat=q_flat, ch8=ch8,
                         gidx_c=gidx_c)

    if bad.any():
        sq = q_flat[bad]
        sb_ = np.searchsorted(q_starts, sq, side="right") - 1
        for bi in np.unique(sb_):
            qsel = sq[sb_ == bi]
            refs = ref_bxyz[r_starts[bi]:r_ends[bi]]
            for s in range(0, len(qsel), 4096):
                part = qsel[s:s + 4096]
                e_ref[part] = r_starts[bi] + _np_exact_rows(
                    query_bxyz[part], refs)
    _CACHE["n_suspect"] = int(bad.sum())
    _CACHE["e_obs"] = e_obs

    return finish(e_ref.reshape(-1))


# revision 15
# speedup vs baseline: 1.2297x; 1.2297x over previous
"""KNN graph kernel for Trainium2 (8 NeuronCores, Bass/Tile).

Problem: per-batch 32-NN of 16384 queries against 16384 refs (B~4 batches,
both sorted by batch id).  Output matches the jax reference:
  e_ref  [M*32] int32  - nearest ref indices, ascending distance per query
  e_query[M*32] int32  - repeat(arange(M), 32)
  mask   [M*32] bool   - (q_z - r_z) >= -1e-5 per edge

Strategy (spatial windowing + quantized score/index packing):
  * Queries are grouped into 136 spatial cells of <=128 (per-batch y/z
    quantile split).  Each 128-query block only scans refs inside its
    cell's (y,z) bounding box expanded by RM=15 (<=1016 refs instead of
    the whole ~4096-ref batch), stride-spread over the window so every
    127-col chunk sees a uniform share.  A query's true 32NN ball fits in
    the window unless it pokes past a non-domain-clipped side; the host
    detects that exactly per query/axis/side and repairs.
  * The tensor engine (fp32r) computes PSUM = T - c*d2 with T = 1.5*2^23:
    every PSUM value lands in [2^23, 2^24) where fp32 accumulation forces
    integer rounding, i.e. the score is quantized to 1/c for free.  ACT
    copies PSUM to SBUF subtracting T (exact); Pool + DVE add idx/128
    (column-in-chunk index; exact while |m| <= 2^17 keeps the 24-bit
    total representable).  Packed values carry the quantized score AND
    the column index, so stage 1 is a SINGLE DVE max8 per chunk - no
    max_index pass, no on-device stage 2.
  * Host: decode candidates, rescore them exactly in f32 (same formula
    family as the reference), merge to top-32.  Rows whose chunk top-8
    may conceal a member get an exact window-level rescore (cheap);
    near-ties and window violations get the full-batch exact recompute.
"""

import numpy as np

K = 32
P = 128            # queries per block (SBUF partitions)
CHUNK = 127
NCHUNK = 8
WDAT = NCHUNK * CHUNK   # 1016 data cols
W = 1024                # padded window width (2 PSUM banks)
HALF = WDAT // 2        # 508 cols per half (one PSUM bank each)
NCAND = NCHUNK * 8      # 64
N_CORES = 8
NBLK = 17               # blocks per core -> 136 cells total
RM = 15.0               # window margin (covering-radius guarantee)
CSC = 256.0             # score scale: quantum = 1/CSC in d2 units
TBIG = 1.5 * 2**23      # 12582912; PSUM offset forcing integer quantization
MVALID = -131072.0      # candidates with m < MVALID (d2 > 512) are discarded
TAU_ORDER = 0.012       # adjacent-gap margin for exact-order trust (d2 units)
PAD_RR = 1.0e6          # |r|^2 for pad columns -> huge negative score
DVE_COLS = 90           # trailing cols per half packed by DVE (engine balance)

_CACHE = {}


def _np_exact_rows(q_rows_bxyz, ref_bxyz):
    """Reference-exact (f32) top-K ref indices for the given query rows."""
    rb, rx = ref_bxyz[:, 0], ref_bxyz[:, 1:4]
    qb, qx = q_rows_bxyz[:, 0], q_rows_bxyz[:, 1:4]
    d2 = (np.sum(qx * qx, axis=1)[:, None]
          + np.sum(rx * rx, axis=1)[None, :]
          - np.float32(2.0) * (qx @ rx.T)).astype(np.float32)
    d2[qb[:, None] != rb[None, :]] = np.inf
    return np.argsort(d2, axis=1, kind="stable")[:, :K].astype(np.int32)


def _np_fallback(ref_bxyz, query_bxyz):
    M = query_bxyz.shape[0]
    e_ref = np.empty((M, K), np.int32)
    step = 2048
    for s in range(0, M, step):
        e_ref[s:s + step] = _np_exact_rows(query_bxyz[s:s + step], ref_bxyz)
    return e_ref.reshape(-1)


def _build_program():
    import concourse.mybir as mybir
    import concourse.tile as tile
    from concourse import bacc

    nc = bacc.Bacc("TRN2", target_bir_lowering=False, debug=False, num_devices=1)
    f32, f32r = mybir.dt.float32, mybir.dt.float32r
    Copy = mybir.ActivationFunctionType.Copy

    # qrs packs the transposed queries (first P cols) and the ref slab
    qrs = nc.dram_tensor("qrs", [NBLK, 5, P + W], f32r, kind="ExternalInput").ap()
    idxc = nc.dram_tensor("idxc", [P, W], f32, kind="ExternalInput").ap()
    c_val = nc.dram_tensor("c_val", [NBLK * P, NCAND], f32, kind="ExternalOutput").ap()

    with tile.TileContext(nc) as tc:
        with tc.tile_pool(name="const", bufs=1) as cpool, \
             tc.tile_pool(name="qp", bufs=4) as qpool, \
             tc.tile_pool(name="tp", bufs=4) as tpool, \
             tc.tile_pool(name="kp", bufs=4) as kpool, \
             tc.tile_pool(name="cvp", bufs=4) as cvpool, \
             tc.tile_pool(name="ps", bufs=6, space="PSUM") as ppool:
            idxrow = cpool.tile([P, W], f32)
            nc.sync.dma_start(out=idxrow[:], in_=idxc)

            for blk in range(NBLK):
                qr = qpool.tile([5, P + W], f32r)
                nc.sync.dma_start(out=qr[:], in_=qrs[blk])

                t = tpool.tile([P, WDAT], f32)
                pk = kpool.tile([P, WDAT], f32)
                cv = cvpool.tile([P, NCAND], f32)
                # two independent halves pipeline matmul/ACT/pack/max8
                for h in range(2):
                    lo = HALF * h
                    ps = ppool.tile([P, HALF], f32)
                    nc.tensor.matmul(ps[:], qr[:, :P], qr[:, P + lo:P + lo + HALF],
                                     start=True, stop=True)
                    nc.scalar.activation(t[:, lo:lo + HALF], ps[:],
                                         Copy, bias=-float(TBIG), scale=1.0)
                    csp = lo + HALF - DVE_COLS
                    nc.gpsimd.tensor_add(pk[:, lo:csp], t[:, lo:csp],
                                         idxrow[:, lo:csp])
                    nc.vector.tensor_add(pk[:, csp:lo + HALF],
                                         t[:, csp:lo + HALF],
                                         idxrow[:, csp:lo + HALF])
                    for s in range(4 * h, 4 * (h + 1)):
                        nc.vector.max(out=cv[:, 8 * s:8 * s + 8],
                                      in_=pk[:, CHUNK * s:CHUNK * (s + 1)])
                nc.sync.dma_start(out=c_val[blk * P:(blk + 1) * P], in_=cv[:])
    nc.compile()
    return nc


def _apportion(total, weights):
    """Split integer `total` proportionally to weights (largest remainder)."""
    w = np.asarray(weights, dtype=np.float64)
    if w.sum() <= 0:
        out = np.zeros(len(w), np.int64)
        if len(w):
            out[0] = total
        return out
    want = w / w.sum() * total
    out = np.floor(want).astype(np.int64)
    rem = int(total - out.sum())
    order = np.argsort(-(want - out))
    out[order[:rem]] += 1
    return out


def _plan_cells(nq_per_batch):
    """Apportion NBLK*N_CORES cells across batches, >= ceil(nq/P) each."""
    total = NBLK * N_CORES
    nq = np.asarray(nq_per_batch, dtype=np.int64)
    mins = -(-nq // P)
    if mins.sum() > total:
        return None
    extra = total - int(mins.sum())
    add = _apportion(extra, nq) if extra > 0 else np.zeros(len(nq), np.int64)
    ncells = mins + add
    ncells[nq == 0] = 0
    return ncells


def kernel(ref_bxyz: np.ndarray, query_bxyz: np.ndarray):
    ref_bxyz = np.ascontiguousarray(ref_bxyz, dtype=np.float32)
    query_bxyz = np.ascontiguousarray(query_bxyz, dtype=np.float32)
    M = query_bxyz.shape[0]
    e_query = np.repeat(np.arange(M, dtype=np.int32), K)

    def finish(e_ref_flat):
        direction = query_bxyz[e_query, 3] - ref_bxyz[e_ref_flat, 3]
        return e_ref_flat, e_query, (direction >= np.float32(-1e-5))

    rb, qb = ref_bxyz[:, 0], query_bxyz[:, 0]
    bids = np.unique(np.concatenate([rb, qb]))
    ok = (len(bids) <= NBLK * N_CORES
          and np.all(np.diff(rb) >= 0) and np.all(np.diff(qb) >= 0))
    if ok:
        r_starts = np.searchsorted(rb, bids, side="left")
        r_ends = np.searchsorted(rb, bids, side="right")
        q_starts = np.searchsorted(qb, bids, side="left")
        q_ends = np.searchsorted(qb, bids, side="right")
        nq_b = q_ends - q_starts
        nr_b = r_ends - r_starts
        ok = bool(np.all((nr_b >= K) | (nq_b == 0)))
        ncells = _plan_cells(nq_b) if ok else None
        ok = ok and ncells is not None
    if not ok:
        return finish(_np_fallback(ref_bxyz, query_bxyz))

    # ---- host prep: spatial cells, windows, slabs ----
    nb = len(bids)
    total_blocks = NBLK * N_CORES
    qrs_in = np.empty((total_blocks, 5, P + W), np.float32)
    qrs_in[:, 0:3, :] = 0.0
    qrs_in[:, 3, :] = np.float32(-CSC)   # qT lane 3
    qrs_in[:, 4, :P] = np.float32(TBIG)  # pad queries: q~ = 0
    qrs_in[:, 3, P:] = PAD_RR            # slab lane 3 pad
    qrs_in[:, 4, P:] = 1.0               # slab ones lane

    win_idx = np.full((total_blocks, W), -1, np.int64)
    rm_blk = np.full(total_blocks, np.float64(RM))
    blk_q = [None] * total_blocks  # global query indices per block
    blk_box = np.zeros((total_blocks, 4), np.float64)  # ymin,ymax,zmin,zmax
    blk_bb = np.zeros((total_blocks, 4), np.float64)   # batch ref y/z bounds

    blk = 0
    cell_overflow = False
    for bi in range(nb):
        n_b = int(ncells[bi])
        if n_b == 0:
            continue
        qs_, qe_ = int(q_starts[bi]), int(q_ends[bi])
        rs_, re_ = int(r_starts[bi]), int(r_ends[bi])
        q_idx = np.arange(qs_, qe_)
        qy, qz = query_bxyz[qs_:qe_, 2], query_bxyz[qs_:qe_, 3]
        r_xyz = ref_bxyz[rs_:re_, 1:4]
        ry, rz = r_xyz[:, 1], r_xyz[:, 2]
        cx0 = float(r_xyz[:, 0].min() + r_xyz[:, 0].max()) / 2

        gy = 4 if n_b >= 8 else 1
        gz_per = _apportion(n_b, np.ones(gy))
        mq_per = _apportion(len(q_idx), gz_per)
        y_order = np.argsort(qy, kind="stable")
        gstart = 0
        for g in range(gy):
            gsel = y_order[gstart:gstart + int(mq_per[g])]
            gstart += int(mq_per[g])
            if len(gsel) == 0:
                continue
            z_order = gsel[np.argsort(qz[gsel], kind="stable")]
            for part in np.array_split(z_order, int(gz_per[g])):
                if len(part) == 0:
                    continue
                if len(part) > P:
                    cell_overflow = True
                    break
                cq = q_idx[part]  # global query ids, cell order
                cy, cz = qy[part], qz[part]
                ymin, ymax = float(cy.min()), float(cy.max())
                zmin, zmax = float(cz.min()), float(cz.max())
                need_y = np.maximum(ymin - ry, ry - ymax)
                need_z = np.maximum(zmin - rz, rz - zmax)
                need = np.maximum(np.maximum(need_y, need_z), 0.0)
                sel = np.nonzero(need <= RM)[0]
                rmb = RM
                if len(sel) > WDAT:
                    kept = np.argpartition(need, WDAT - 1)[:WDAT]
                    sel = np.sort(kept)
                    rmb = float(need[sel].max()) - 1e-4
                nw = len(sel)
                # stride-spread the window over all WDAT cols so each
                # chunk sees ~nw/NCHUNK refs (keeps chunk top-8 losses rare)
                spread = (np.arange(nw, dtype=np.int64) * WDAT) // max(nw, 1)
                win_idx[blk, spread] = rs_ + sel
                rm_blk[blk] = rmb
                blk_q[blk] = cq
                blk_box[blk] = (ymin, ymax, zmin, zmax)
                if re_ > rs_:
                    blk_bb[blk] = (ry.min(), ry.max(), rz.min(), rz.max())
                c0 = np.array([cx0, (ymin + ymax) / 2, (zmin + zmax) / 2],
                              np.float64)
                rt = (r_xyz[sel].astype(np.float64) - c0).astype(np.float32)
                qrs_in[blk, 0:3, P + spread] = rt
                qrs_in[blk, 3, P + spread] = np.sum(
                    rt.astype(np.float64) ** 2, axis=1).astype(np.float32)
                qt_ = (query_bxyz[cq, 1:4].astype(np.float64) - c0).astype(
                    np.float32)
                nv = len(cq)
                qrs_in[blk, 0:3, :nv] = (2.0 * CSC) * qt_.T
                qrs_in[blk, 4, :nv] = (
                    TBIG - CSC * np.sum(qt_.astype(np.float64) ** 2, axis=1)
                ).astype(np.float32)
                blk += 1
            if cell_overflow:
                break
        if cell_overflow:
            break
    if cell_overflow:
        return finish(_np_fallback(ref_bxyz, query_bxyz))

    # ---- device ----
    if "nc" not in _CACHE:
        _CACHE["nc"] = _build_program()
    nc = _CACHE["nc"]
    from concourse.bass_utils import run_bass_kernel_spmd
    qrs_c = qrs_in.reshape(N_CORES, NBLK, 5, P + W)
    idxc = np.broadcast_to(
        ((np.arange(W) % CHUNK) / 128.0).astype(np.float32), (P, W)).copy()
    in_maps = [{"qrs": qrs_c[c], "idxc": idxc} for c in range(N_CORES)]
    _CACHE["last_in_maps"] = in_maps
    res = run_bass_kernel_spmd(nc, in_maps, list(range(N_CORES)))
    _CACHE["last_results"] = res

    cv = np.concatenate([res.results[c]["c_val"] for c in range(N_CORES)],
                        axis=0).reshape(total_blocks, P, NCAND)

    # ---- host post: decode, rescore, merge, repair ----
    blocks_list = [i for i in range(total_blocks) if blk_q[i] is not None]
    bsel = np.concatenate([np.full(len(blk_q[i]), i, np.int64)
                           for i in blocks_list])
    rsel = np.concatenate([np.arange(len(blk_q[i]), dtype=np.int64)
                           for i in blocks_list])
    q_flat = np.concatenate([blk_q[i] for i in blocks_list])  # [M] global qids

    p = cv[bsel, rsel].astype(np.float64)              # [M, NCAND]
    m = np.floor(p)
    idxl = np.rint((p - m) * 128.0).astype(np.int64)
    pos = (np.arange(NCAND) // 8)[None, :] * CHUNK + np.clip(idxl, 0, CHUNK - 1)
    valid = (m >= MVALID) & (idxl < CHUNK)
    d2q = -m / CSC                                     # device-quantized d2
    gidx = win_idx[bsel[:, None], pos]
    valid &= gidx >= 0
    gidx_c = np.where(valid, gidx, 0)

    qx_all = query_bxyz[:, 1:4]
    rx_all = ref_bxyz[:, 1:4]
    qq_all = np.sum(qx_all * qx_all, axis=1)           # f32, reference formula
    rr_all = np.sum(rx_all * rx_all, axis=1)

    dot = np.einsum("qd,qkd->qk", qx_all[q_flat], rx_all[gidx_c],
                    dtype=np.float32, casting="same_kind")
    d2x = (qq_all[q_flat][:, None] + rr_all[gidx_c]
           - np.float32(2.0) * dot).astype(np.float64)
    d2x[~valid] = np.inf

    near = valid & (d2q < 500.0)
    e_obs = float(np.abs(np.where(near, d2q - d2x, 0.0)).max())
    if e_obs > 1.0:
        return finish(_np_fallback(ref_bxyz, query_bxyz))
    kappa = 2.0 * e_obs + 2.0 / CSC + 1e-3

    order = np.lexsort((gidx_c, d2x), axis=-1)
    top = order[:, :K + 1]
    d2_sorted = np.take_along_axis(d2x, top, axis=1)
    ref_sorted = np.take_along_axis(gidx_c, top, axis=1)

    e_ref = np.empty((M, K), np.int32)
    e_ref[q_flat] = ref_sorted[:, :K].astype(np.int32)

    nvalid = valid.sum(axis=1)
    d2_32 = d2_sorted[:, K - 1]
    b_nv = nvalid < K + 1
    # per-query, per-axis-side window containment: the d_32 ball must fit in
    # the (y,z)-expanded cell box, except on sides where the window already
    # reaches the batch's ref extent (domain-clipped side: nothing to miss).
    d32r = np.sqrt(np.where(np.isfinite(d2_32), d2_32, 0.0)) + 1e-3
    qy_f = query_bxyz[q_flat, 2].astype(np.float64)
    qz_f = query_bxyz[q_flat, 3].astype(np.float64)
    box = blk_box[bsel]
    bb = blk_bb[bsel]
    rmb = rm_blk[bsel]
    okq = ((qy_f - d32r >= box[:, 0] - rmb) | (box[:, 0] - rmb <= bb[:, 0]))
    okq &= ((qy_f + d32r <= box[:, 1] + rmb) | (box[:, 1] + rmb >= bb[:, 1]))
    okq &= ((qz_f - d32r >= box[:, 2] - rmb) | (box[:, 2] - rmb <= bb[:, 2]))
    okq &= ((qz_f + d32r <= box[:, 3] + rmb) | (box[:, 3] + rmb >= bb[:, 3]))
    b_vio = (~okq) | ~np.isfinite(d2_32)
    b_tie = (np.diff(d2_sorted, axis=1) < TAU_ORDER).any(axis=1)
    ch8 = d2q[:, 7::8]
    b_con = (valid[:, 7::8] & (ch8 <= d2_32[:, None] + kappa)).any(axis=1)
    full = b_nv | b_vio | b_tie
    con_only = b_con & ~full
    _CACHE["sus"] = dict(nv=int(b_nv.sum()), vio=int(b_vio.sum()),
                         tie=int(b_tie.sum()), con=int(b_con.sum()),
                         con_only=int(con_only.sum()),
                         e_obs=e_obs, kappa=kappa)

    # tier-1 repair: exact rescore over the row's full window (handles
    # chunk-top8 concealment; needs containment, which holds: not b_vio)
    if con_only.any():
        esc = []
        for i in np.unique(bsel[con_only]):
            rows = np.nonzero(con_only & (bsel == i))[0]
            wmask = win_idx[i] >= 0
            wg = win_idx[i][wmask]
            qsel = q_flat[rows]
            if len(wg) < K + 1:
                esc.append(qsel)
                continue
            d2w = (qq_all[qsel][:, None] + rr_all[wg]
                   - np.float32(2.0)
                   * (qx_all[qsel] @ rx_all[wg].T.astype(np.float32))
                   ).astype(np.float64)
            od = np.lexsort((np.broadcast_to(wg, d2w.shape), d2w), axis=-1)
            tops = od[:, :K + 1]
            d2s = np.take_along_axis(d2w, tops, axis=1)
            e_ref[qsel] = wg[tops[:, :K]].astype(np.int32)
            tie = (np.diff(d2s, axis=1) < TAU_ORDER).any(axis=1)
            if tie.any():
                esc.append(qsel[tie])
        if esc:
            full_extra = np.concatenate(esc)
        else:
            full_extra = np.empty(0, np.int64)
    else:
        full_extra = np.empty(0, np.int64)

    # tier-2 repair: reference-exact full-batch recompute
    sq = np.concatenate([q_flat[full], full_extra])
    if len(sq):
        sq = np.unique(sq)
        sb_ = np.searchsorted(q_starts, sq, side="right") - 1
        for bi in np.unique(sb_):
            qsel = sq[sb_ == bi]
            refs = ref_bxyz[r_starts[bi]:r_ends[bi]]
            for s in range(0, len(qsel), 4096):
                part = qsel[s:s + 4096]
                e_ref[part] = r_starts[bi] + _np_exact_rows(
                    query_bxyz[part], refs)
    _CACHE["n_suspect"] = int(len(sq))
    _CACHE["dbg"] = dict(win_idx=win_idx, rm_blk=rm_blk, bsel=bsel,
                         d2_32=d2_32, d2q=d2q, valid=valid, nvalid=nvalid,
                         d2_sorted=d2_sorted, q_flat=q_flat, ch8=ch8)

    return finish(e_ref.reshape(-1))
